# revision 5
# baseline (speedup 1.0000x reference)
"""Trainium2 Bass kernel for nn_DMLoss (Chamfer polygon loss) — quadratic-argmin
rewrite.

Sharding: data-parallel over batch B=32 across 8 cores (4 batches/core); each
core emits [128, 12] partial sums combined on host (same contract as the
original kernel).

pred2gt: for each pred, the distance to the interp points of gt segment i is a
quadratic in t; the discrete argmin over t in {0..9} is the grid point nearest
the parabola vertex.  Per (pred, seg) we evaluate the quadratic at the rounded
clamped vertex via two fp16 hi/lo split matmuls (m1 = -beta/1280,
m2 = -gamma'/128, exact to ~0.03 in key units) + 6 elementwise ops, rank segs
with MAX8, then exactly refine the top-2 segments at t in {tlo, tlo+1}
(bit-exact reference arithmetic) after a 32B-row indirect gather of per-seg
data.

gt2pred: exact squared distances via ACT Square with per-partition bias on
partition-broadcast pred rows; negate + MAX8/FI8 = exact first-index argmin;
winner row gathered from a pred_polys_ table.
"""

import os
import sys

for _p in ("/opt/trn_rl_repo", "/root/.axon_site/_ro/trn_rl_repo"):
    if os.path.isdir(_p) and _p not in sys.path:
        sys.path.insert(0, _p)

import numpy as np

import concourse.bass as bass
import concourse.bacc as bacc
import concourse.mybir as mybir
from concourse.bass import IndirectOffsetOnAxis
from concourse.bass_utils import run_bass_kernel_spmd
from concourse.tile import TileContext
from concourse.tile_rust import add_dep_helper

F32 = mybir.dt.float32
F16 = mybir.dt.float16
U32 = mybir.dt.uint32
AF = mybir.ActivationFunctionType
ALU = mybir.AluOpType
AX = mybir.AxisListType

B, NP, NG, T = 32, 512, 512, 10
NCORES = 8
BLOC = B // NCORES
NCH = NP // 128            # 4 chunks of 128 (preds and gts)
KC = 2                     # candidate segments refined per pred

C2 = float(3.0 * 2.0**22)          # 1.5*2^23: RNE integer rounding incl negatives
ULP9 = float(5.9604644775390625e-08)


def build_nc():
    nc = bacc.Bacc()

    ini = nc.dram_tensor("ini_pred_poly", [BLOC, NP, 2], F32, kind="ExternalInput")
    pred2 = nc.dram_tensor("pred_polys_", [BLOC, NP, 2], F32, kind="ExternalInput")
    gt = nc.dram_tensor("gt_polys", [BLOC, NG, 2], F32, kind="ExternalInput")
    kmask = nc.dram_tensor("keyPointsMask", [BLOC, NG], F32, kind="ExternalInput")
    id16 = nc.dram_tensor("id16", [128, 128], F16, kind="ExternalInput")
    out = nc.dram_tensor("out", [128, 12], F32, kind="ExternalOutput")

    segtabs = [nc.dram_tensor(f"segtab{b_}", [NG, 8], F32) for b_ in range(BLOC)]
    ptabs = [nc.dram_tensor(f"ptab{b_}", [NP, 2], F32) for b_ in range(BLOC)]

    with TileContext(nc) as tc:
        with (
            tc.tile_pool(name="const", bufs=1) as cpool,
            tc.tile_pool(name="setup", bufs=2) as stp,
            tc.tile_pool(name="rows16", bufs=2) as r16,
            tc.tile_pool(name="bcp", bufs=2) as bcp,
            tc.tile_pool(name="ev", bufs=2) as ev,
            tc.tile_pool(name="g2p", bufs=2) as g2p,
            tc.tile_pool(name="ref", bufs=2) as ref,
            tc.tile_pool(name="pt16", bufs=2, space="PSUM") as pt16,
            tc.tile_pool(name="km1", bufs=1, space="PSUM") as km1,
            tc.tile_pool(name="km2", bufs=1, space="PSUM") as km2,
        ):
            ident16 = cpool.tile([128, 128], F16)
            nc.sync.dma_start(out=ident16[:], in_=id16[:])
            res = cpool.tile([128, 12], F32)

            for b_ in range(BLOC):
                # ============ per-batch setup ============
                gsb = stp.tile([128, NCH, 2], F32, tag="gsb")
                nc.sync.dma_start(
                    out=gsb[:], in_=gt[b_][:].rearrange("(m p) c -> p m c", p=128))
                grsb = stp.tile([128, NCH, 2], F32, tag="grsb")
                nc.sync.dma_start(
                    out=grsb[1:128, :, :],
                    in_=gt[b_][:].rearrange("(m p) c -> p m c", p=128)[0:127, :, :])
                nc.sync.dma_start(out=grsb[0:1, 1:4, :],
                                  in_=gt[b_:b_ + 1, 127:384:128, :])
                nc.sync.dma_start(out=grsb[0:1, 0:1, :],
                                  in_=gt[b_:b_ + 1, NG - 1:NG, :])
                pred2_b = stp.tile([128, NCH, 2], F32, tag="pred2_b")
                nc.sync.dma_start(
                    out=pred2_b[:],
                    in_=pred2[b_][:].rearrange("(m p) c -> p m c", p=128))
                pxy = stp.tile([128, NCH, 2], F32, tag="pxy")
                nc.sync.dma_start(
                    out=pxy[:], in_=ini[b_][:].rearrange("(m p) c -> p m c", p=128))
                mask_b = stp.tile([128, NCH], F32, tag="mask_b")
                nc.sync.dma_start(
                    out=mask_b[:], in_=kmask[b_][:].rearrange("(c p) -> p c", p=128))

                ptw = nc.sync.dma_start(
                    out=ptabs[b_][:].rearrange("(m p) c -> p m c", p=128),
                    in_=pred2_b[:])

                # ---- stage math (chunk-major, [128,4,*] tiles) ----
                dg = stp.tile([128, NCH, 2], F32, tag="dg")
                nc.gpsimd.tensor_tensor(out=dg[:], in0=gsb[:], in1=grsb[:],
                                        op=ALU.subtract)
                sqgr = stp.tile([128, NCH, 2], F32, tag="sqgr")
                nc.gpsimd.tensor_tensor(out=sqgr[:], in0=grsb[:], in1=grsb[:],
                                        op=ALU.mult)
                w4 = stp.tile([128, NCH], F32, tag="w4")
                nc.gpsimd.tensor_tensor(out=w4[:], in0=sqgr[:, :, 0],
                                        in1=sqgr[:, :, 1], op=ALU.add)
                pg = stp.tile([128, NCH, 2], F32, tag="pg")
                nc.gpsimd.tensor_tensor(out=pg[:], in0=gsb[:], in1=grsb[:],
                                        op=ALU.mult)
                v4 = stp.tile([128, NCH], F32, tag="v4")
                nc.gpsimd.tensor_tensor(out=v4[:], in0=pg[:, :, 0], in1=pg[:, :, 1],
                                        op=ALU.add)
                vw4 = stp.tile([128, NCH], F32, tag="vw4")
                nc.gpsimd.tensor_tensor(out=vw4[:], in0=v4[:], in1=w4[:],
                                        op=ALU.subtract)
                sqdg = stp.tile([128, NCH, 2], F32, tag="sqdg")
                nc.vector.tensor_tensor(out=sqdg[:], in0=dg[:], in1=dg[:],
                                        op=ALU.mult)
                al4 = stp.tile([128, NCH], F32, tag="al4")
                nc.vector.tensor_tensor(out=al4[:], in0=sqdg[:, :, 0],
                                        in1=sqdg[:, :, 1], op=ALU.add)
                rc4 = stp.tile([128, NCH], F32, tag="rc4")
                nc.vector.reciprocal(out=rc4[:], in_=al4[:])

                # gather table rows: [gx, gy, gxr, gyr, 10/alpha, alpha, 0, 0]
                ra10 = stp.tile([128, NCH], F32, tag="ra10")
                nc.gpsimd.tensor_scalar(out=ra10[:], in0=rc4[:], scalar1=10.0,
                                        scalar2=None, op0=ALU.mult)
                stws = [
                    nc.sync.dma_start(
                        out=segtabs[b_][:, 0:2].rearrange("(m p) c -> p m c", p=128),
                        in_=gsb[:]),
                    nc.sync.dma_start(
                        out=segtabs[b_][:, 2:4].rearrange("(m p) c -> p m c", p=128),
                        in_=grsb[:]),
                    nc.sync.dma_start(
                        out=segtabs[b_][:, 4:5].rearrange("(m p) c -> p m c", p=128),
                        in_=ra10[:].unsqueeze(2)),
                    nc.sync.dma_start(
                        out=segtabs[b_][:, 5:6].rearrange("(m p) c -> p m c", p=128),
                        in_=al4[:].unsqueeze(2)),
                ]

                ngsb = stp.tile([128, NCH, 2], F32, tag="ngsb")
                nc.gpsimd.tensor_scalar(out=ngsb[:], in0=gsb[:], scalar1=-1.0,
                                        scalar2=None, op0=ALU.mult)

                # ---- fp16 hi/lo stages ----
                # rhsA (m1 = -beta/1280 rows): [r0h r0h r1h r1h r0l r0l r1l r1l r2h r2l]
                # r0 = 0.2*dgx/16, r1 = 0.2*dgy/16, r2 = -0.2*vw/16
                m1f = stp.tile([128, NCH, 2], F32, tag="m1f")
                nc.gpsimd.tensor_scalar(out=m1f[:], in0=dg[:], scalar1=0.0125,
                                        scalar2=None, op0=ALU.mult)
                m1c = stp.tile([128, NCH], F32, tag="m1c")
                nc.gpsimd.tensor_scalar(out=m1c[:], in0=vw4[:], scalar1=-0.0125,
                                        scalar2=None, op0=ALU.mult)
                stA = r16.tile([128, NCH, 10], F16, tag="stA")
                bA = m1f[:].unsqueeze(2).to_broadcast([128, NCH, 2, 2])
                nc.vector.tensor_copy(
                    out=stA[:, :, 0:4].rearrange("p m (r c) -> p m r c", r=2),
                    in_=bA)
                nc.vector.tensor_tensor(
                    out=stA[:, :, 4:8].rearrange("p m (r c) -> p m r c", r=2),
                    in0=bA,
                    in1=stA[:, :, 0:4].rearrange("p m (r c) -> p m r c", r=2),
                    op=ALU.subtract)
                nc.vector.tensor_copy(out=stA[:, :, 8], in_=m1c[:])
                nc.vector.tensor_tensor(out=stA[:, :, 9], in0=m1c[:],
                                        in1=stA[:, :, 8], op=ALU.subtract)

                # rhsB (m2 = -gamma'/128 rows): r3 = 2*gxr/16, r4 = 2*gyr/16, r5 = -w/16
                m2f = stp.tile([128, NCH, 2], F32, tag="m2f")
                nc.gpsimd.tensor_scalar(out=m2f[:], in0=grsb[:], scalar1=0.125,
                                        scalar2=None, op0=ALU.mult)
                m2c = stp.tile([128, NCH], F32, tag="m2c")
                nc.gpsimd.tensor_scalar(out=m2c[:], in0=w4[:], scalar1=-0.0625,
                                        scalar2=None, op0=ALU.mult)
                stB = r16.tile([128, NCH, 10], F16, tag="stB")
                bB = m2f[:].unsqueeze(2).to_broadcast([128, NCH, 2, 2])
                nc.vector.tensor_copy(
                    out=stB[:, :, 0:4].rearrange("p m (r c) -> p m r c", r=2),
                    in_=bB)
                nc.vector.tensor_tensor(
                    out=stB[:, :, 4:8].rearrange("p m (r c) -> p m r c", r=2),
                    in0=bB,
                    in1=stB[:, :, 0:4].rearrange("p m (r c) -> p m r c", r=2),
                    op=ALU.subtract)
                nc.vector.tensor_copy(out=stB[:, :, 8], in_=m2c[:])
                nc.vector.tensor_tensor(out=stB[:, :, 9], in0=m2c[:],
                                        in1=stB[:, :, 8], op=ALU.subtract)

                # t-block rows: rhsA rows scaled per-seg by 6400/alpha (exact h/l split)
                raq = stp.tile([128, NCH], F32, tag="raq")
                nc.gpsimd.tensor_scalar(out=raq[:], in0=rc4[:], scalar1=6400.0,
                                        scalar2=None, op0=ALU.mult)
                m1fq = stp.tile([128, NCH, 2], F32, tag="m1fq")
                nc.gpsimd.tensor_tensor(
                    out=m1fq[:], in0=m1f[:],
                    in1=raq[:].unsqueeze(2).to_broadcast([128, NCH, 2]),
                    op=ALU.mult)
                nc.gpsimd.tensor_scalar(out=m1fq[:], in0=m1fq[:], scalar1=60000.0,
                                        scalar2=-60000.0, op0=ALU.min, op1=ALU.max)
                m1cq = stp.tile([128, NCH], F32, tag="m1cq")
                nc.gpsimd.tensor_tensor(out=m1cq[:], in0=m1c[:], in1=raq[:],
                                        op=ALU.mult)
                nc.gpsimd.tensor_scalar(out=m1cq[:], in0=m1cq[:], scalar1=60000.0,
                                        scalar2=-60000.0, op0=ALU.min, op1=ALU.max)
                stT = r16.tile([128, NCH, 10], F16, tag="stT")
                bT = m1fq[:].unsqueeze(2).to_broadcast([128, NCH, 2, 2])
                nc.vector.tensor_copy(
                    out=stT[:, :, 0:4].rearrange("p m (r c) -> p m r c", r=2),
                    in_=bT)
                nc.vector.tensor_tensor(
                    out=stT[:, :, 4:8].rearrange("p m (r c) -> p m r c", r=2),
                    in0=bT,
                    in1=stT[:, :, 0:4].rearrange("p m (r c) -> p m r c", r=2),
                    op=ALU.subtract)
                nc.vector.tensor_copy(out=stT[:, :, 8], in_=m1cq[:])
                nc.vector.tensor_tensor(out=stT[:, :, 9], in0=m1cq[:],
                                        in1=stT[:, :, 8], op=ALU.subtract)

                # lhsT stage: cols [pxh pxl pyh pyl pxh pxl pyh pyl c c], c=0.125
                px8 = stp.tile([128, NCH, 2], F32, tag="px8")
                nc.gpsimd.tensor_scalar(out=px8[:], in0=pxy[:], scalar1=0.125,
                                        scalar2=None, op0=ALU.mult)
                stL = r16.tile([128, NCH, 10], F16, tag="stL")
                nc.vector.tensor_copy(out=stL[:, :, 0:2], in_=px8[:])
                nc.vector.tensor_tensor(out=stL[:, :, 2:4], in0=px8[:],
                                        in1=stL[:, :, 0:2], op=ALU.subtract)
                nc.vector.tensor_copy(out=stL[:, :, 4:8], in_=stL[:, :, 0:4])
                nc.vector.memset(stL[:, :, 8:10], 0.125)

                # ---- PE transposes -> fp16 row tiles ----
                rhsA = r16.tile([10, NP], F16, tag="rhsA")
                rhsB = r16.tile([10, NP], F16, tag="rhsB")
                rhsT = r16.tile([10, NP], F16, tag="rhsT")
                lhsT10 = r16.tile([10, NP], F16, tag="lhsT10")
                for src, dst in ((stA, rhsA), (stB, rhsB), (stT, rhsT),
                                 (stL, lhsT10)):
                    ps = pt16.tile([10, NP], F16, tag="pt")
                    for m in range(NCH):
                        nc.tensor.transpose(ps[:, 128 * m:128 * (m + 1)],
                                            in_=src[:, m, :], identity=ident16[:])
                    nc.scalar.activation(out=dst[:], in_=ps[:], func=AF.Copy)

                # ---- replicated rows via readback + partition_broadcast ----
                # bc cols: [alpha/12800 | 6400/alpha | px | py]
                arow = stp.tile([1, 2048], F32, tag="arow")
                rb1 = nc.sync.dma_start(
                    out=arow[0:1, 0:512],
                    in_=segtabs[b_][:, 5:6].rearrange("s one -> one s"))
                rb2 = nc.sync.dma_start(
                    out=arow[0:1, 512:1024],
                    in_=segtabs[b_][:, 4:5].rearrange("s one -> one s"))
                add_dep_helper(rb1.ins, stws[3].ins, sync=True,
                               reason="alpha row readback waits on segtab write")
                add_dep_helper(rb2.ins, stws[2].ins, sync=True,
                               reason="ra row readback waits on segtab write")
                nc.vector.tensor_scalar(out=arow[0:1, 0:512], in0=arow[0:1, 0:512],
                                        scalar1=float(1.0 / 12800.0), scalar2=None,
                                        op0=ALU.mult)
                nc.vector.tensor_scalar(out=arow[0:1, 512:1024],
                                        in0=arow[0:1, 512:1024],
                                        scalar1=640.0, scalar2=None, op0=ALU.mult)
                nc.sync.dma_start(out=arow[0:1, 1024:1536], in_=ini[b_:b_ + 1, :, 0])
                nc.sync.dma_start(out=arow[0:1, 1536:2048], in_=ini[b_:b_ + 1, :, 1])
                bc = bcp.tile([128, 2048], F32, tag="bc")
                nc.gpsimd.partition_broadcast(bc[:], arow[:])

                # ============ pred2gt: 2 groups of 2 chunks ============
                idx8s = []
                cseg = ref.tile([128, NCH, KC, 8], F32, tag="cseg")
                gathers = []
                for g in range(2):
                    psm1 = km1.tile([128, 1024], F32, tag="m1")
                    psm2 = km2.tile([128, 1024], F32, tag="m2")
                    psmT = km2.tile([128, 1024], F32, tag="mt")
                    for j in range(2):
                        c = 2 * g + j
                        sl = slice(128 * c, 128 * (c + 1))
                        cols = slice(512 * j, 512 * (j + 1))
                        nc.tensor.matmul(psm1[:, cols], lhsT=lhsT10[:, sl],
                                         rhs=rhsA[:], start=True, stop=True)
                        nc.tensor.matmul(psm2[:, cols], lhsT=lhsT10[:, sl],
                                         rhs=rhsB[:], start=True, stop=True)
                        nc.tensor.matmul(psmT[:, cols], lhsT=lhsT10[:, sl],
                                         rhs=rhsT[:], start=True, stop=True)

                    def rep2(col0):
                        return bc[:, col0:col0 + 512].unsqueeze(1).to_broadcast(
                            [128, 2, 512])

                    v2 = lambda tl: tl.rearrange("p (two s) -> p two s", two=2)

                    tc_ = ev.tile([128, 1024], F32, tag="tc")
                    nc.vector.tensor_scalar(out=tc_[:], in0=psmT[:], scalar1=C2,
                                            scalar2=C2, op0=ALU.add,
                                            op1=ALU.subtract)
                    nc.vector.tensor_scalar(out=tc_[:], in0=tc_[:], scalar1=0.0,
                                            scalar2=9.0, op0=ALU.max, op1=ALU.min)
                    q1 = ev.tile([128, 1024], F32, tag="q1")
                    nc.vector.tensor_tensor(out=v2(q1), in0=v2(tc_), in1=rep2(0),
                                            op=ALU.mult)
                    q2 = ev.tile([128, 1024], F32, tag="q2")
                    nc.vector.tensor_tensor(out=q2[:], in0=psm1[:], in1=q1[:],
                                            op=ALU.subtract)
                    q3 = ev.tile([128, 1024], F32, tag="q3")
                    nc.vector.tensor_tensor(out=q3[:], in0=q2[:], in1=tc_[:],
                                            op=ALU.mult)
                    key = ev.tile([128, 1024], F32, tag="key")
                    nc.vector.tensor_tensor(out=key[:], in0=q3[:], in1=psm2[:],
                                            op=ALU.add)
                    for j in range(2):
                        c = 2 * g + j
                        cols = slice(512 * j, 512 * (j + 1))
                        mx8 = ref.tile([128, 8], F32, tag=f"mx8_{c}")
                        idx8 = ref.tile([128, 8], U32, tag=f"idx8_{c}")
                        nc.vector.max(out=mx8[:], in_=key[:, cols])
                        nc.vector.max_index(out=idx8[:], in_max=mx8[:],
                                            in_values=key[:, cols])
                        idx8s.append(idx8)
                        for k in range(KC):
                            gth = nc.gpsimd.indirect_dma_start(
                                out=cseg[:, c, k, :], out_offset=None,
                                in_=segtabs[b_][:],
                                in_offset=IndirectOffsetOnAxis(
                                    ap=idx8[:, k:k + 1], axis=0))
                            gathers.append(gth)
                for gth in gathers:
                    for _w in stws:
                        add_dep_helper(gth.ins, _w.ins, sync=True,
                                       reason="seg gather waits on segtab write")

                # ---- exact refine of KC*2 interp candidates ----
                cg = cseg[:, :, :, 0:2]
                cgr = cseg[:, :, :, 2:4]
                cdg = ref.tile([128, NCH, KC, 2], F32, tag="cdg")
                nc.vector.tensor_tensor(out=cdg[:], in0=cg, in1=cgr, op=ALU.subtract)
                pd = ref.tile([128, NCH, KC, 2], F32, tag="pd")
                nc.gpsimd.tensor_tensor(
                    out=pd[:], in0=pxy[:].unsqueeze(2).to_broadcast(
                        [128, NCH, KC, 2]), in1=cgr, op=ALU.subtract)
                dqx = ref.tile([128, NCH, KC], F32, tag="dqx")
                nc.vector.tensor_tensor(out=dqx[:], in0=pd[:, :, :, 0],
                                        in1=cdg[:, :, :, 0], op=ALU.mult)
                dqy = ref.tile([128, NCH, KC], F32, tag="dqy")
                nc.gpsimd.tensor_tensor(out=dqy[:], in0=pd[:, :, :, 1],
                                        in1=cdg[:, :, :, 1], op=ALU.mult)
                dot = ref.tile([128, NCH, KC], F32, tag="dot")
                nc.vector.tensor_tensor(out=dot[:], in0=dqx[:], in1=dqy[:],
                                        op=ALU.add)
                tau = ref.tile([128, NCH, KC], F32, tag="tau")
                nc.gpsimd.tensor_tensor(out=tau[:], in0=dot[:],
                                        in1=cseg[:, :, :, 4], op=ALU.mult)
                tcd = ref.tile([128, NCH, KC, 2], F32, tag="tcd")
                nc.vector.tensor_scalar(out=tcd[:, :, :, 0], in0=tau[:],
                                        scalar1=float(C2 - 0.5), scalar2=C2,
                                        op0=ALU.add, op1=ALU.subtract)
                nc.vector.tensor_scalar(out=tcd[:, :, :, 0], in0=tcd[:, :, :, 0],
                                        scalar1=0.0, scalar2=8.0, op0=ALU.max,
                                        op1=ALU.min)
                nc.vector.tensor_scalar(out=tcd[:, :, :, 1], in0=tcd[:, :, :, 0],
                                        scalar1=1.0, scalar2=None, op0=ALU.add)
                av = ref.tile([128, NCH, KC, 2], F32, tag="av")
                nc.gpsimd.tensor_scalar(out=av[:], in0=tcd[:], scalar1=0.1,
                                        scalar2=None, op0=ALU.mult)
                corr = ref.tile([128, NCH, KC, 2], F32, tag="corr")
                nc.gpsimd.tensor_scalar(out=corr[:], in0=tcd[:], scalar1=9.0,
                                        scalar2=ULP9, op0=ALU.is_equal, op1=ALU.mult)
                nc.gpsimd.tensor_tensor(out=av[:], in0=av[:], in1=corr[:],
                                        op=ALU.subtract)
                bv = ref.tile([128, NCH, KC, 2], F32, tag="bv")
                nc.gpsimd.tensor_scalar(out=bv[:], in0=av[:], scalar1=-1.0,
                                        scalar2=1.0, op0=ALU.mult, op1=ALU.add)
                KB = [128, NCH, KC, 2]
                ix1 = ref.tile(KB, F32, tag="ix1")
                nc.vector.tensor_tensor(out=ix1[:],
                                        in0=cg[:, :, :, 0:1].to_broadcast(KB),
                                        in1=av[:], op=ALU.mult)
                ix2 = ref.tile(KB, F32, tag="ix2")
                nc.gpsimd.tensor_tensor(out=ix2[:],
                                        in0=cgr[:, :, :, 0:1].to_broadcast(KB),
                                        in1=bv[:], op=ALU.mult)
                ix = ref.tile(KB, F32, tag="ix")
                nc.vector.tensor_tensor(out=ix[:], in0=ix1[:], in1=ix2[:],
                                        op=ALU.add)
                iy1 = ref.tile(KB, F32, tag="iy1")
                nc.gpsimd.tensor_tensor(out=iy1[:],
                                        in0=cg[:, :, :, 1:2].to_broadcast(KB),
                                        in1=av[:], op=ALU.mult)
                iy2 = ref.tile(KB, F32, tag="iy2")
                nc.vector.tensor_tensor(out=iy2[:],
                                        in0=cgr[:, :, :, 1:2].to_broadcast(KB),
                                        in1=bv[:], op=ALU.mult)
                iy = ref.tile(KB, F32, tag="iy")
                nc.gpsimd.tensor_tensor(out=iy[:], in0=iy1[:], in1=iy2[:],
                                        op=ALU.add)
                ddx = ref.tile(KB, F32, tag="ddx")
                nc.vector.tensor_tensor(
                    out=ddx[:], in0=ix[:],
                    in1=pxy[:, :, 0:1].unsqueeze(3).to_broadcast(KB),
                    op=ALU.subtract)
                ddy = ref.tile(KB, F32, tag="ddy")
                nc.gpsimd.tensor_tensor(
                    out=ddy[:], in0=iy[:],
                    in1=pxy[:, :, 1:2].unsqueeze(3).to_broadcast(KB),
                    op=ALU.subtract)
                sqx = ref.tile(KB, F32, tag="sqx")
                nc.vector.tensor_tensor(out=sqx[:], in0=ddx[:], in1=ddx[:],
                                        op=ALU.mult)
                sqy = ref.tile(KB, F32, tag="sqy")
                nc.gpsimd.tensor_tensor(out=sqy[:], in0=ddy[:], in1=ddy[:],
                                        op=ALU.mult)
                dd = ref.tile(KB, F32, tag="dd")
                nc.vector.tensor_tensor(out=dd[:], in0=sqx[:], in1=sqy[:],
                                        op=ALU.add)
                ddv = dd.rearrange("p m k t -> p m (k t)")
                dmin = ref.tile([128, NCH], F32, tag="dmin")
                nc.vector.tensor_reduce(out=dmin[:], in_=ddv, axis=AX.X, op=ALU.min)
                sel = ref.tile(KB, F32, tag="sel")
                nc.vector.tensor_tensor(
                    out=sel.rearrange("p m k t -> p m (k t)"), in0=ddv,
                    in1=dmin[:].unsqueeze(2).to_broadcast([128, NCH, KC * 2]),
                    op=ALU.is_equal)
                sx = ref.tile(KB, F32, tag="sx")
                nc.vector.tensor_tensor(out=sx[:], in0=sel[:], in1=ix[:],
                                        op=ALU.mult)
                sy = ref.tile(KB, F32, tag="sy")
                nc.gpsimd.tensor_tensor(out=sy[:], in0=sel[:], in1=iy[:],
                                        op=ALU.mult)
                df = ref.tile([128, NCH, 2], F32, tag="df")
                nc.vector.tensor_reduce(out=df[:, :, 0],
                                        in_=sx.rearrange("p m k t -> p m (k t)"),
                                        axis=AX.X, op=ALU.add)
                nc.vector.tensor_reduce(out=df[:, :, 1],
                                        in_=sy.rearrange("p m k t -> p m (k t)"),
                                        axis=AX.X, op=ALU.add)
                nc.vector.tensor_tensor(out=df[:], in0=pred2_b[:], in1=df[:],
                                        op=ALU.subtract)
                nc.vector.tensor_reduce(out=res[:, b_:b_ + 1], in_=df[:],
                                        axis=AX.XY, op=ALU.add,
                                        apply_absolute_value=True)

                # ============ gt2pred (exact) ============
                npred = g2p.tile([128, NCH, 2], F32, tag="npred")
                g2 = []
                for g in range(2):
                    sq1 = g2p.tile([128, 1024], F32, tag="sq1")
                    sq2 = g2p.tile([128, 1024], F32, tag="sq2")
                    for j in range(2):
                        c = 2 * g + j
                        cols = slice(512 * j, 512 * (j + 1))
                        nc.scalar.activation(out=sq1[:, cols], in_=bc[:, 1024:1536],
                                             func=AF.Square, bias=ngsb[:, c, 0:1])
                        nc.scalar.activation(out=sq2[:, cols], in_=bc[:, 1536:2048],
                                             func=AF.Square, bias=ngsb[:, c, 1:2])
                    d2t = g2p.tile([128, 1024], F32, tag="d2t")
                    nc.vector.tensor_tensor(out=d2t[:], in0=sq1[:], in1=sq2[:],
                                            op=ALU.add)
                    key2 = g2p.tile([128, 1024], F32, tag="key2")
                    nc.scalar.activation(out=key2[:], in_=d2t[:], func=AF.Copy,
                                         scale=-1.0)
                    for j in range(2):
                        c = 2 * g + j
                        cols = slice(512 * j, 512 * (j + 1))
                        mxb = g2p.tile([128, 8], F32, tag=f"mxb_{c}")
                        ixb = g2p.tile([128, 8], U32, tag=f"ixb_{c}")
                        nc.vector.max(out=mxb[:], in_=key2[:, cols])
                        nc.vector.max_index(out=ixb[:], in_max=mxb[:],
                                            in_values=key2[:, cols])
                        gth = nc.gpsimd.indirect_dma_start(
                            out=npred[:, c, :], out_offset=None, in_=ptabs[b_][:],
                            in_offset=IndirectOffsetOnAxis(ap=ixb[:, 0:1], axis=0))
                        g2.append(gth)
                for gth in g2:
                    add_dep_helper(gth.ins, ptw.ins, sync=True,
                                   reason="pred gather waits on ptab write")

                md = g2p.tile([128, NCH, 2], F32, tag="md")
                nc.vector.tensor_tensor(out=md[:], in0=npred[:], in1=gsb[:],
                                        op=ALU.subtract)
                sabs = g2p.tile([128, NCH], F32, tag="sabs")
                nc.vector.tensor_reduce(out=sabs[:], in_=md[:], axis=AX.X,
                                        op=ALU.add, apply_absolute_value=True)
                smask = g2p.tile([128, NCH], F32, tag="smask")
                nc.vector.tensor_tensor(out=smask[:], in0=sabs[:], in1=mask_b[:],
                                        op=ALU.mult)
                nc.vector.tensor_reduce(out=res[:, 4 + b_:5 + b_], in_=smask[:],
                                        axis=AX.X, op=ALU.add)
                nc.vector.tensor_reduce(out=res[:, 8 + b_:9 + b_], in_=mask_b[:],
                                        axis=AX.X, op=ALU.add)

            nc.sync.dma_start(out=out[:], in_=res[:])

    nc.compile()
    return nc


_NC_CACHE = None


def _get_nc():
    global _NC_CACHE
    if _NC_CACHE is None:
        _NC_CACHE = build_nc()
    return _NC_CACHE


def make_in_maps(ini_pred_poly, pred_polys_, gt_polys, keyPointsMask):
    id16 = np.eye(128, dtype=np.float16)
    in_maps = []
    for i in range(NCORES):
        s = slice(BLOC * i, BLOC * (i + 1))
        in_maps.append({
            "ini_pred_poly": np.ascontiguousarray(ini_pred_poly[s], dtype=np.float32),
            "pred_polys_": np.ascontiguousarray(pred_polys_[s], dtype=np.float32),
            "gt_polys": np.ascontiguousarray(gt_polys[s], dtype=np.float32),
            "keyPointsMask": np.ascontiguousarray(keyPointsMask[s], dtype=np.float32),
            "id16": id16,
        })
    return in_maps


def combine_outputs(outs):
    acc = np.zeros(12, dtype=np.float64)
    for o in outs:
        acc += o.astype(np.float64).sum(axis=0)
    s_p2g = acc[0:4].sum()
    s_g2p = acc[4:8].sum()
    s_msk = 2.0 * acc[8:12].sum()
    loss_pred2gt = s_p2g / (B * NP * 2)
    loss = (s_g2p / (s_msk + 1.0) + loss_pred2gt) / 2.0
    return np.float32(loss)


def kernel(ini_pred_poly, pred_polys_, gt_polys, keyPointsMask):
    nc = _get_nc()
    in_maps = make_in_maps(ini_pred_poly, pred_polys_, gt_polys, keyPointsMask)
    r = run_bass_kernel_spmd(nc, in_maps, list(range(NCORES)))
    return combine_outputs([r.results[i]["out"] for i in range(NCORES)])


if __name__ == "__main__":
    import reference

    inputs = {k: np.asarray(v) for k, v in reference.setup_inputs().items()}
    got = kernel(**inputs)
    print("kernel loss:", got)
